# revision 17
# baseline (speedup 1.0000x reference)
"""Distributed Trainium2 Bass kernel for 16-head attention.

Reference op: B=2, S=2048, D=1024, H=16 multi-head attention with an
elementwise 0/1 mask, computed as
    out = softmax(mask((q Wq^T)(k Wk^T)^T / sqrt(64))) (v Wv^T) Wo^T

Sharding over 8 NeuronCores: core c handles batch c//4 and head group
c%4 (4 heads = 256 channels). Attention is computed fully locally in a
"dual" layout (scores transposed, [k, q]); the context is exchanged
with one small AllGather per 512-query tile inside each 4-core batch
group, and the output projection is split along the OUTPUT feature dim
(each core holds a 256-column slice of Wo^T), so the host-side unshard
is a pure concatenation.

Compute dtype bf16 (TensorE 1 cyc/row), accumulation f32 in PSUM.
"""

import sys
import types

sys.path.insert(0, "/opt/trn_rl_repo")

import numpy as np
import ml_dtypes

BF16 = ml_dtypes.bfloat16

B = 2
S = 2048
DM = 1024
DL = 256  # d-model slice per core (4 heads)
HL = 4  # heads per core
DK = 64
P = 128
QT_N = 4  # query tiles of 512
QTS = 512
KC = 16  # key chunks of 128
MC = 8  # contraction chunks of 128 over d_model
GROUPS = [[0, 1, 2, 3], [4, 5, 6, 7]]

_cached = {}


def _build():
    import concourse.bass as bass
    import concourse.mybir as mybir
    from concourse import bacc
    from concourse.tile import TileContext

    fp32 = mybir.dt.float32
    bf16 = mybir.dt.bfloat16

    nc = bacc.Bacc(num_devices=8)

    qT = nc.dram_tensor("qT", [DM, S], bf16, kind="ExternalInput")
    kT = nc.dram_tensor("kT", [DM, S], bf16, kind="ExternalInput")
    vT = nc.dram_tensor("vT", [DM, S], bf16, kind="ExternalInput")
    maskT = nc.dram_tensor("maskT", [S, S], bf16, kind="ExternalInput")
    wq = nc.dram_tensor("wq", [DM, DL], bf16, kind="ExternalInput")
    wk = nc.dram_tensor("wk", [DM, DL], bf16, kind="ExternalInput")
    wv = nc.dram_tensor("wv", [DM, DL], bf16, kind="ExternalInput")
    wo = nc.dram_tensor("wo", [DM, DL], bf16, kind="ExternalInput")
    y = nc.dram_tensor("y", [S, DL], fp32, kind="ExternalOutput")

    cc_in = [
        [
            nc.dram_tensor(f"cc_in{t}_{p}", [P, QTS], bf16, kind="Internal")
            for p in range(2)
        ]
        for t in range(QT_N)
    ]
    cc_out = [
        [
            nc.dram_tensor(f"cc_out{t}_{p}", [4 * P, QTS], bf16, kind="Internal")
            for p in range(2)
        ]
        for t in range(QT_N)
    ]

    with TileContext(nc) as tc:
        with (
            tc.tile_pool(name="xT", bufs=10) as xT_pool,
            tc.tile_pool(name="w", bufs=32) as w_pool,
            tc.tile_pool(name="qkt", bufs=2) as qkt_pool,
            tc.tile_pool(name="vext", bufs=16) as vext_pool,
            tc.tile_pool(name="mask", bufs=2) as mask_pool,
            tc.tile_pool(name="attn", bufs=2) as attn_pool,
            tc.tile_pool(name="sm", bufs=6) as sm_pool,
            tc.tile_pool(name="ctxn", bufs=8) as ctxn_pool,
            tc.tile_pool(name="ctxg", bufs=4) as ctxg_pool,
            tc.tile_pool(name="ysb", bufs=4) as y_pool,
            tc.tile_pool(name="ps_big", bufs=2, space="PSUM") as ps_big,
            tc.tile_pool(name="ps_acc", bufs=2, space="PSUM") as ps_acc,
            tc.tile_pool(name="ps_out", bufs=2, space="PSUM") as ps_out,
        ):
            # ---- weight loads -------------------------------------------------
            def load_w(dram):
                tiles = []
                for m in range(MC):
                    t_ = w_pool.tile([P, DL], bf16, tag="w")
                    nc.sync.dma_start(t_[:], dram[P * m : P * (m + 1), :])
                    tiles.append(t_)
                return tiles

            # ---- Q/K projections: out QT/KT [256, 2048] as 2 tiles [128,2048]
            def proj_T(x_dram, w_sb, tag):
                x_sb = []
                for m in range(MC):
                    t_ = xT_pool.tile([P, S], bf16, tag="xT")
                    nc.sync.dma_start(t_[:], x_dram[P * m : P * (m + 1), :])
                    x_sb.append(t_)
                out_tiles = []
                for dt in range(2):
                    ot = qkt_pool.tile([P, S], bf16, tag=tag)
                    for st in range(4):
                        ps = ps_acc.tile([P, QTS], fp32, tag="acc")
                        for m in range(MC):
                            nc.tensor.matmul(
                                ps[:],
                                w_sb[m][:, P * dt : P * (dt + 1)],
                                x_sb[m][:, QTS * st : QTS * (st + 1)],
                                start=(m == 0),
                                stop=(m == MC - 1),
                            )
                        nc.vector.tensor_copy(
                            ot[:, QTS * st : QTS * (st + 1)], ps[:]
                        )
                    out_tiles.append(ot)
                return out_tiles

            wq_sb = load_w(wq)
            QT_sb = proj_T(qT, wq_sb, "QT")
            wk_sb = load_w(wk)
            KT_sb = proj_T(kT, wk_sb, "KT")
            wv_sb = load_w(wv)

            # ---- V projection -> V_ext tiles [128, 4*65] ([V_h | 1] blocks)
            vT_sb = []
            for m in range(MC):
                t_ = xT_pool.tile([P, S], bf16, tag="xT")
                nc.sync.dma_start(t_[:], vT[P * m : P * (m + 1), :])
                vT_sb.append(t_)
            vext = []
            for st in range(KC):
                ps = ps_acc.tile([P, QTS], fp32, tag="acc")
                for m in range(MC):
                    nc.tensor.matmul(
                        ps[:, 0:DL],
                        vT_sb[m][:, P * st : P * (st + 1)],
                        wv_sb[m][:],
                        start=(m == 0),
                        stop=(m == MC - 1),
                    )
                ve = vext_pool.tile([P, HL * (DK + 1)], bf16, tag="vext")
                nc.vector.memset(ve[:], 1.0)
                for h in range(HL):
                    nc.vector.tensor_copy(
                        ve[:, 65 * h : 65 * h + DK],
                        ps[:, DK * h : DK * (h + 1)],
                    )
                vext.append(ve)

            wo_sb = load_w(wo)

            def load_mask(t):
                mt_ = mask_pool.tile(
                    [P, KC * QTS], bf16, tag="mask", name=f"mask{t}"
                )
                src3 = maskT.rearrange("(kc p) q -> p kc q", p=P)[
                    :, :, QTS * t : QTS * (t + 1)
                ]
                dst3 = mt_[:].rearrange("p (kc q) -> p kc q", q=QTS)
                nc.sync.dma_start(dst3, src3)
                return mt_

            mts = {0: load_mask(0)}

            ones_lhs = sm_pool.tile([DK + 1, P], bf16, tag="ones")
            nc.vector.memset(ones_lhs[:], 1.0)

            # ---- attention + exchange + output projection per query tile ----
            # The exchange readback + output projection for query tile t are
            # issued inside tile t+1's block so the AllGather latency hides
            # under the next tile's attention and never head-of-line-blocks
            # an engine queue.
            def do_readback(t):
                ctxg = []
                for p in range(2):
                    cg = ctxg_pool.tile(
                        [P, 4 * QTS], bf16, tag="ctxg", name=f"cg{t}_{p}"
                    )
                    src3 = cc_out[t][p].rearrange("(i pp) q -> pp i q", pp=P)
                    dst3 = cg[:].rearrange("pp (i q) -> pp i q", q=QTS)
                    nc.sync.dma_start(dst3, src3)
                    ctxg.append(cg)
                return ctxg

            def do_outproj(t, ctxg, qs_list=(0, 1, 2, 3)):
                # d-chunk dc (global 128 rows) lives in pair dc%2's gather
                # output at block dc//2.
                for qs in qs_list:
                    op = ps_out.tile(
                        [P, DL], fp32, tag="out", name=f"op{t}_{qs}"
                    )
                    for dc in range(MC):
                        src = ctxg[dc % 2][
                            :, QTS * (dc // 2) + P * qs : QTS * (dc // 2) + P * (qs + 1)
                        ]
                        nc.tensor.matmul(
                            op[:],
                            src,
                            wo_sb[dc][:],
                            start=(dc == 0),
                            stop=(dc == MC - 1),
                        )
                    ys = y_pool.tile([P, DL], fp32, tag="ysb", name=f"ys{t}_{qs}")
                    nc.vector.tensor_copy(ys[:], op[:])
                    r = QTS * t + P * qs
                    nc.sync.dma_start(y[r : r + P, :], ys[:])

            for t in range(QT_N):
                mt = mts.pop(t)
                if t + 1 < QT_N:
                    mts[t + 1] = load_mask(t + 1)

                ctxn_tiles = []
                for pair in range(2):
                    at = {}
                    cp = {}
                    for h01 in range(2):
                        h = 2 * pair + h01
                        at[h] = attn_pool.tile(
                            [P, KC * QTS], bf16, tag="attn", name=f"at{h}_{t}"
                        )
                        cp[h] = ps_acc.tile(
                            [P, QTS], fp32, tag="acc", name=f"cp{h}_{t}"
                        )
                    # fused pipeline: per 2-chunk group, scores -> exp ->
                    # mask-mult; ctx accumulation runs 2 groups behind so PE
                    # never waits on the exp/mask chain.
                    def ctx_group(grp):
                        for j in range(2):
                            kc = 2 * grp + j
                            for h01 in range(2):
                                h = 2 * pair + h01
                                nc.tensor.matmul(
                                    cp[h][0 : DK + 1, :],
                                    vext[kc][:, 65 * h : 65 * h + DK + 1],
                                    at[h][:, QTS * kc : QTS * (kc + 1)],
                                    start=(kc == 0),
                                    stop=(kc == KC - 1),
                                )

                    for grp in range(KC // 2):
                        if grp >= 2:
                            ctx_group(grp - 2)
                        sp = {}
                        for h01 in range(2):
                            h = 2 * pair + h01
                            sp[h] = ps_big.tile(
                                [P, 1024], fp32, tag="big",
                                name=f"sp{h}_{t}_{grp}",
                            )
                        for j in range(2):
                            kc = 2 * grp + j
                            for h01 in range(2):
                                h = 2 * pair + h01
                                r0 = DK * h01
                                nc.tensor.matmul(
                                    sp[h][:, QTS * j : QTS * (j + 1)],
                                    KT_sb[pair][
                                        r0 : r0 + DK, P * kc : P * (kc + 1)
                                    ],
                                    QT_sb[pair][
                                        r0 : r0 + DK, QTS * t : QTS * (t + 1)
                                    ],
                                    start=True,
                                    stop=True,
                                    tile_position=(r0, 0),
                                )
                        gsl = slice(1024 * grp, 1024 * (grp + 1))
                        for h01 in range(2):
                            h = 2 * pair + h01
                            nc.scalar.activation(
                                at[h][:, gsl],
                                sp[h][:],
                                mybir.ActivationFunctionType.Exp,
                            )
                            nc.vector.tensor_mul(
                                at[h][:, gsl], at[h][:, gsl], mt[:, gsl]
                            )
                    ctx_group(KC // 2 - 2)
                    ctx_group(KC // 2 - 1)
                    # softmax normalization: bf16 sums row -> PE broadcast
                    # matmul (ones[1,128] from row 64) -> DVE recip -> DVE mul
                    for h01 in range(2):
                        h = 2 * pair + h01
                        srow = sm_pool.tile(
                            [DK + 1, QTS], bf16, tag="srow", name=f"srow{h}_{t}"
                        )
                        nc.vector.tensor_copy(
                            srow[DK : DK + 1, :], cp[h][DK : DK + 1, :]
                        )
                        bc = ps_out.tile(
                            [P, QTS], fp32, tag="out", name=f"bc{h}_{t}"
                        )
                        nc.tensor.matmul(
                            bc[:],
                            ones_lhs[DK : DK + 1, :],
                            srow[DK : DK + 1, :],
                            start=True,
                            stop=True,
                            tile_position=(DK, 0),
                        )
                        recipb = sm_pool.tile(
                            [P, QTS], fp32, tag="recipb", name=f"recipb{h}_{t}"
                        )
                        nc.vector.reciprocal_approx_fast(
                            out=recipb[:], in_=bc[:]
                        )
                        cn = ctxn_pool.tile(
                            [DK, QTS], bf16, tag="ctxn", name=f"cn{h}_{t}"
                        )
                        nc.vector.tensor_mul(
                            cn[:], cp[h][0:DK, :], recipb[0:DK, :]
                        )
                        ctxn_tiles.append(cn)
                    # exchange this pair's ctx^T as soon as it is normalized
                    for h01 in range(2):
                        h = 2 * pair + h01
                        nc.sync.dma_start(
                            cc_in[t][pair][DK * h01 : DK * (h01 + 1), :],
                            ctxn_tiles[h][:],
                        )
                    if pair == 0 and t > 0:
                        ctxg_prev = do_readback(t - 1)
                    nc.gpsimd.collective_compute(
                        "AllGather",
                        mybir.AluOpType.bypass,
                        replica_groups=GROUPS,
                        ins=[cc_in[t][pair][:]],
                        outs=[cc_out[t][pair][:]],
                    )
                    if t > 0:
                        do_outproj(t - 1, ctxg_prev, (0, 1) if pair == 0 else (2, 3))
            do_outproj(QT_N - 1, do_readback(QT_N - 1))

    nc.compile()
    return nc


def _get_nc():
    if "nc" not in _cached:
        _cached["nc"] = _build()
    return _cached["nc"]


def _shard_inputs(q, k, v, mask, w_q, w_k, w_v, w_o):
    in_maps = []
    scale = 1.0 / np.sqrt(DK)
    wqT = (w_q.astype(np.float64) * scale).astype(np.float32).T  # [DM, DM]
    wkT = w_k.T
    wvT = w_v.T
    woT = w_o.T
    for c in range(8):
        b, g = c // 4, c % 4
        sl = slice(DL * g, DL * (g + 1))
        in_maps.append(
            {
                "qT": np.ascontiguousarray(q[b].T).astype(BF16),
                "kT": np.ascontiguousarray(k[b].T).astype(BF16),
                "vT": np.ascontiguousarray(v[b].T).astype(BF16),
                "maskT": np.ascontiguousarray(mask[b].T).astype(BF16),
                "wq": np.ascontiguousarray(wqT[:, sl]).astype(BF16),
                "wk": np.ascontiguousarray(wkT[:, sl]).astype(BF16),
                "wv": np.ascontiguousarray(wvT[:, sl]).astype(BF16),
                "wo": np.ascontiguousarray(woT[:, sl]).astype(BF16),
            }
        )
    return in_maps


def kernel(q, k, v, mask, w_q, w_k, w_v, w_o, _trace=False, _tmpdir=None):
    from concourse import bass_utils

    nc = _get_nc()
    in_maps = _shard_inputs(q, k, v, mask, w_q, w_k, w_v, w_o)
    res = bass_utils.run_bass_kernel_spmd(
        nc,
        in_maps,
        core_ids=list(range(8)),
        trace=_trace,
        tmpdir=_tmpdir,
    )
    out = np.empty((B, S, DM), dtype=np.float32)
    for c in range(8):
        b, g = c // 4, c % 4
        out[b, :, DL * g : DL * (g + 1)] = res.results[c]["y"]
    if _trace:
        _cached["last_exec_time_ns"] = res.exec_time_ns
        _cached["last_results"] = res
    return out


# revision 19
# speedup vs baseline: 1.0049x; 1.0049x over previous
"""Distributed Trainium2 Bass kernel for 16-head attention.

Reference op: B=2, S=2048, D=1024, H=16 multi-head attention with an
elementwise 0/1 mask, computed as
    out = softmax(mask((q Wq^T)(k Wk^T)^T / sqrt(64))) (v Wv^T) Wo^T

Sharding over 8 NeuronCores: core c handles batch c//4 and head group
c%4 (4 heads = 256 channels). Attention is computed fully locally in a
"dual" layout (scores transposed, [k, q]); the context is exchanged
with one small AllGather per 512-query tile inside each 4-core batch
group, and the output projection is split along the OUTPUT feature dim
(each core holds a 256-column slice of Wo^T), so the host-side unshard
is a pure concatenation.

Compute dtype bf16 (TensorE 1 cyc/row), accumulation f32 in PSUM.
"""

import sys
import types

sys.path.insert(0, "/opt/trn_rl_repo")

import numpy as np
import ml_dtypes

BF16 = ml_dtypes.bfloat16

B = 2
S = 2048
DM = 1024
DL = 256  # d-model slice per core (4 heads)
HL = 4  # heads per core
DK = 64
P = 128
QT_N = 4  # query tiles of 512
QTS = 512
KC = 16  # key chunks of 128
MC = 8  # contraction chunks of 128 over d_model
GROUPS = [[0, 1, 2, 3], [4, 5, 6, 7]]

_cached = {}


def _build():
    import concourse.bass as bass
    import concourse.mybir as mybir
    from concourse import bacc
    from concourse.tile import TileContext

    fp32 = mybir.dt.float32
    bf16 = mybir.dt.bfloat16

    nc = bacc.Bacc(num_devices=8)

    qT = nc.dram_tensor("qT", [DM, S], bf16, kind="ExternalInput")
    kT = nc.dram_tensor("kT", [DM, S], bf16, kind="ExternalInput")
    vT = nc.dram_tensor("vT", [DM, S], bf16, kind="ExternalInput")
    maskT = nc.dram_tensor("maskT", [S, S], bf16, kind="ExternalInput")
    wq = nc.dram_tensor("wq", [DM, DL], bf16, kind="ExternalInput")
    wk = nc.dram_tensor("wk", [DM, DL], bf16, kind="ExternalInput")
    wv = nc.dram_tensor("wv", [DM, DL], bf16, kind="ExternalInput")
    wo = nc.dram_tensor("wo", [DM, DL], bf16, kind="ExternalInput")
    y = nc.dram_tensor("y", [S, DL], fp32, kind="ExternalOutput")

    cc_in = [
        [
            nc.dram_tensor(f"cc_in{t}_{p}", [P, QTS], bf16, kind="Internal")
            for p in range(2)
        ]
        for t in range(QT_N)
    ]
    cc_out = [
        [
            nc.dram_tensor(f"cc_out{t}_{p}", [4 * P, QTS], bf16, kind="Internal")
            for p in range(2)
        ]
        for t in range(QT_N)
    ]

    with TileContext(nc) as tc:
        with (
            tc.tile_pool(name="xT", bufs=10) as xT_pool,
            tc.tile_pool(name="w", bufs=32) as w_pool,
            tc.tile_pool(name="qkt", bufs=2) as qkt_pool,
            tc.tile_pool(name="vext", bufs=16) as vext_pool,
            tc.tile_pool(name="mask", bufs=2) as mask_pool,
            tc.tile_pool(name="attn", bufs=2) as attn_pool,
            tc.tile_pool(name="sm", bufs=6) as sm_pool,
            tc.tile_pool(name="ctxn", bufs=8) as ctxn_pool,
            tc.tile_pool(name="ctxg", bufs=4) as ctxg_pool,
            tc.tile_pool(name="ysb", bufs=4) as y_pool,
            tc.tile_pool(name="ps_big", bufs=2, space="PSUM") as ps_big,
            tc.tile_pool(name="ps_acc", bufs=2, space="PSUM") as ps_acc,
            tc.tile_pool(name="ps_out", bufs=2, space="PSUM") as ps_out,
        ):
            # ---- weight loads -------------------------------------------------
            def load_w(dram):
                tiles = []
                for m in range(MC):
                    t_ = w_pool.tile([P, DL], bf16, tag="w")
                    nc.sync.dma_start(t_[:], dram[P * m : P * (m + 1), :])
                    tiles.append(t_)
                return tiles

            # ---- Q/K projections: out QT/KT [256, 2048] as 2 tiles [128,2048]
            def proj_T(x_dram, w_sb, tag, split_first=False):
                x_sb = []
                for m in range(MC):
                    t_ = xT_pool.tile([P, S], bf16, tag="xT")
                    if split_first:
                        for cq in range(4):
                            nc.sync.dma_start(
                                t_[:, QTS * cq : QTS * (cq + 1)],
                                x_dram[P * m : P * (m + 1), QTS * cq : QTS * (cq + 1)],
                            )
                    else:
                        nc.sync.dma_start(t_[:], x_dram[P * m : P * (m + 1), :])
                    x_sb.append(t_)
                out_tiles = []
                for dt in range(2):
                    ot = qkt_pool.tile([P, S], bf16, tag=tag)
                    for st in range(2):
                        ps = ps_big.tile([P, 1024], fp32, tag="big")
                        for m in range(MC):
                            for sh in range(2):
                                nc.tensor.matmul(
                                    ps[:, QTS * sh : QTS * (sh + 1)],
                                    w_sb[m][:, P * dt : P * (dt + 1)],
                                    x_sb[m][
                                        :,
                                        1024 * st + QTS * sh : 1024 * st + QTS * (sh + 1),
                                    ],
                                    start=(m == 0),
                                    stop=(m == MC - 1),
                                )
                        nc.vector.tensor_copy(
                            ot[:, 1024 * st : 1024 * (st + 1)], ps[:]
                        )
                    out_tiles.append(ot)
                return out_tiles

            wq_sb = load_w(wq)
            QT_sb = proj_T(qT, wq_sb, "QT", split_first=True)
            wk_sb = load_w(wk)
            KT_sb = proj_T(kT, wk_sb, "KT")
            wv_sb = load_w(wv)

            # ---- V projection -> V_ext tiles [128, 4*65] ([V_h | 1] blocks)
            vT_sb = []
            for m in range(MC):
                t_ = xT_pool.tile([P, S], bf16, tag="xT")
                nc.sync.dma_start(t_[:], vT[P * m : P * (m + 1), :])
                vT_sb.append(t_)
            vext = []
            for st in range(KC):
                ps = ps_acc.tile([P, QTS], fp32, tag="acc")
                for m in range(MC):
                    nc.tensor.matmul(
                        ps[:, 0:DL],
                        vT_sb[m][:, P * st : P * (st + 1)],
                        wv_sb[m][:],
                        start=(m == 0),
                        stop=(m == MC - 1),
                    )
                ve = vext_pool.tile([P, HL * (DK + 1)], bf16, tag="vext")
                nc.vector.memset(ve[:], 1.0)
                for h in range(HL):
                    nc.vector.tensor_copy(
                        ve[:, 65 * h : 65 * h + DK],
                        ps[:, DK * h : DK * (h + 1)],
                    )
                vext.append(ve)

            wo_sb = load_w(wo)

            def load_mask(t):
                mt_ = mask_pool.tile(
                    [P, KC * QTS], bf16, tag="mask", name=f"mask{t}"
                )
                src3 = maskT.rearrange("(kc p) q -> p kc q", p=P)[
                    :, :, QTS * t : QTS * (t + 1)
                ]
                dst3 = mt_[:].rearrange("p (kc q) -> p kc q", q=QTS)
                nc.sync.dma_start(dst3, src3)
                return mt_

            mts = {0: load_mask(0)}

            ones_lhs = sm_pool.tile([DK + 1, P], bf16, tag="ones")
            nc.vector.memset(ones_lhs[:], 1.0)

            # ---- attention + exchange + output projection per query tile ----
            # The exchange readback + output projection for query tile t are
            # issued inside tile t+1's block so the AllGather latency hides
            # under the next tile's attention and never head-of-line-blocks
            # an engine queue.
            def do_readback(t, pairs=(0, 1)):
                ctxg = []
                for p in pairs:
                    cg = ctxg_pool.tile(
                        [P, 4 * QTS], bf16, tag="ctxg", name=f"cg{t}_{p}"
                    )
                    src3 = cc_out[t][p].rearrange("(i pp) q -> pp i q", pp=P)
                    dst3 = cg[:].rearrange("pp (i q) -> pp i q", q=QTS)
                    nc.sync.dma_start(dst3, src3)
                    ctxg.append(cg)
                return ctxg

            def do_outproj(t, ctxg, qs_list=(0, 1, 2, 3)):
                # d-chunk dc (global 128 rows) lives in pair dc%2's gather
                # output at block dc//2. Pair-0 chunks first so the psum
                # accumulation can start before pair 1's gather lands.
                dcs = [0, 2, 4, 6, 1, 3, 5, 7]
                for qs in qs_list:
                    op = ps_out.tile(
                        [P, DL], fp32, tag="out", name=f"op{t}_{qs}"
                    )
                    for i, dc in enumerate(dcs):
                        src = ctxg[dc % 2][
                            :, QTS * (dc // 2) + P * qs : QTS * (dc // 2) + P * (qs + 1)
                        ]
                        nc.tensor.matmul(
                            op[:],
                            src,
                            wo_sb[dc][:],
                            start=(i == 0),
                            stop=(i == MC - 1),
                        )
                    ys = y_pool.tile([P, DL], fp32, tag="ysb", name=f"ys{t}_{qs}")
                    nc.vector.tensor_copy(ys[:], op[:])
                    r = QTS * t + P * qs
                    nc.sync.dma_start(y[r : r + P, :], ys[:])

            for t in range(QT_N):
                mt = mts.pop(t)
                if t + 1 < QT_N:
                    mts[t + 1] = load_mask(t + 1)

                ctxn_tiles = []
                for pair in range(2):
                    at = {}
                    cp = {}
                    for h01 in range(2):
                        h = 2 * pair + h01
                        at[h] = attn_pool.tile(
                            [P, KC * QTS], bf16, tag="attn", name=f"at{h}_{t}"
                        )
                        cp[h] = ps_acc.tile(
                            [P, QTS], fp32, tag="acc", name=f"cp{h}_{t}"
                        )
                    # fused pipeline: per 2-chunk group, scores -> exp ->
                    # mask-mult; ctx accumulation runs 2 groups behind so PE
                    # never waits on the exp/mask chain.
                    def ctx_group(grp):
                        for j in range(2):
                            kc = 2 * grp + j
                            for h01 in range(2):
                                h = 2 * pair + h01
                                nc.tensor.matmul(
                                    cp[h][0 : DK + 1, :],
                                    vext[kc][:, 65 * h : 65 * h + DK + 1],
                                    at[h][:, QTS * kc : QTS * (kc + 1)],
                                    start=(kc == 0),
                                    stop=(kc == KC - 1),
                                )

                    for grp in range(KC // 2):
                        if grp >= 2:
                            ctx_group(grp - 2)
                        sp = {}
                        for h01 in range(2):
                            h = 2 * pair + h01
                            sp[h] = ps_big.tile(
                                [P, 1024], fp32, tag="big",
                                name=f"sp{h}_{t}_{grp}",
                            )
                        for j in range(2):
                            kc = 2 * grp + j
                            for h01 in range(2):
                                h = 2 * pair + h01
                                r0 = DK * h01
                                nc.tensor.matmul(
                                    sp[h][:, QTS * j : QTS * (j + 1)],
                                    KT_sb[pair][
                                        r0 : r0 + DK, P * kc : P * (kc + 1)
                                    ],
                                    QT_sb[pair][
                                        r0 : r0 + DK, QTS * t : QTS * (t + 1)
                                    ],
                                    start=True,
                                    stop=True,
                                    tile_position=(r0, 0),
                                )
                        gsl = slice(1024 * grp, 1024 * (grp + 1))
                        for h01 in range(2):
                            h = 2 * pair + h01
                            nc.scalar.activation(
                                at[h][:, gsl],
                                sp[h][:],
                                mybir.ActivationFunctionType.Exp,
                            )
                            nc.vector.tensor_mul(
                                at[h][:, gsl], at[h][:, gsl], mt[:, gsl]
                            )
                    ctx_group(KC // 2 - 2)
                    ctx_group(KC // 2 - 1)
                    # softmax normalization: bf16 sums row -> PE broadcast
                    # matmul (ones[1,128] from row 64) -> DVE recip -> DVE mul
                    for h01 in range(2):
                        h = 2 * pair + h01
                        srow = sm_pool.tile(
                            [DK + 1, QTS], bf16, tag="srow", name=f"srow{h}_{t}"
                        )
                        nc.vector.tensor_copy(
                            srow[DK : DK + 1, :], cp[h][DK : DK + 1, :]
                        )
                        bc = ps_out.tile(
                            [P, QTS], fp32, tag="out", name=f"bc{h}_{t}"
                        )
                        nc.tensor.matmul(
                            bc[:],
                            ones_lhs[DK : DK + 1, :],
                            srow[DK : DK + 1, :],
                            start=True,
                            stop=True,
                            tile_position=(DK, 0),
                        )
                        recipb = sm_pool.tile(
                            [P, QTS], fp32, tag="recipb", name=f"recipb{h}_{t}"
                        )
                        nc.vector.reciprocal_approx_fast(
                            out=recipb[:], in_=bc[:]
                        )
                        cn = ctxn_pool.tile(
                            [DK, QTS], bf16, tag="ctxn", name=f"cn{h}_{t}"
                        )
                        nc.vector.tensor_mul(
                            cn[:], cp[h][0:DK, :], recipb[0:DK, :]
                        )
                        ctxn_tiles.append(cn)
                    # exchange this pair's ctx^T as soon as it is normalized
                    for h01 in range(2):
                        h = 2 * pair + h01
                        nc.sync.dma_start(
                            cc_in[t][pair][DK * h01 : DK * (h01 + 1), :],
                            ctxn_tiles[h][:],
                        )
                    if pair == 0 and t > 0:
                        ctxg_prev = do_readback(t - 1)
                    nc.gpsimd.collective_compute(
                        "AllGather",
                        mybir.AluOpType.bypass,
                        replica_groups=GROUPS,
                        ins=[cc_in[t][pair][:]],
                        outs=[cc_out[t][pair][:]],
                    )
                    if t > 0:
                        do_outproj(t - 1, ctxg_prev, (0, 1) if pair == 0 else (2, 3))
            do_outproj(QT_N - 1, do_readback(QT_N - 1))

    nc.compile()
    return nc


def _get_nc():
    if "nc" not in _cached:
        _cached["nc"] = _build()
    return _cached["nc"]


def _shard_inputs(q, k, v, mask, w_q, w_k, w_v, w_o):
    in_maps = []
    scale = 1.0 / np.sqrt(DK)
    wqT = (w_q.astype(np.float64) * scale).astype(np.float32).T  # [DM, DM]
    wkT = w_k.T
    wvT = w_v.T
    woT = w_o.T
    for c in range(8):
        b, g = c // 4, c % 4
        sl = slice(DL * g, DL * (g + 1))
        in_maps.append(
            {
                "qT": np.ascontiguousarray(q[b].T).astype(BF16),
                "kT": np.ascontiguousarray(k[b].T).astype(BF16),
                "vT": np.ascontiguousarray(v[b].T).astype(BF16),
                "maskT": np.ascontiguousarray(mask[b].T).astype(BF16),
                "wq": np.ascontiguousarray(wqT[:, sl]).astype(BF16),
                "wk": np.ascontiguousarray(wkT[:, sl]).astype(BF16),
                "wv": np.ascontiguousarray(wvT[:, sl]).astype(BF16),
                "wo": np.ascontiguousarray(woT[:, sl]).astype(BF16),
            }
        )
    return in_maps


def kernel(q, k, v, mask, w_q, w_k, w_v, w_o, _trace=False, _tmpdir=None):
    from concourse import bass_utils

    nc = _get_nc()
    in_maps = _shard_inputs(q, k, v, mask, w_q, w_k, w_v, w_o)
    res = bass_utils.run_bass_kernel_spmd(
        nc,
        in_maps,
        core_ids=list(range(8)),
        trace=_trace,
        tmpdir=_tmpdir,
    )
    out = np.empty((B, S, DM), dtype=np.float32)
    for c in range(8):
        b, g = c // 4, c % 4
        out[b, :, DL * g : DL * (g + 1)] = res.results[c]["y"]
    if _trace:
        _cached["last_exec_time_ns"] = res.exec_time_ns
        _cached["last_results"] = res
    return out
